# revision 28
# baseline (speedup 1.0000x reference)
"""MoE expert-group kernel for Trainium2 (8 NeuronCores).

Problem: T=2048 tokens, E=8 experts, D=1024, I=2048.
  out[t] = silu(x[t] @ w_gate[e]) * (x[t] @ w_up[e]) @ w_down[e],  e = expert_indices[t]

Strategy: expert parallelism. Host-side (numpy) routing gathers tokens by
expert (this is the "all-to-all"); core e runs expert e's dense
gate/up/silu/down pipeline; host scatters rows back.

On-chip formulation is fully transposed so no transposes are ever needed:
  gateT = Wg^T @ X^T        (stationary = 128x128 Wg block, moving = xT [128, C])
  hidT  = silu(gateT)*upT   (ACT sigmoid + DVE muls, written bf16)
  outT  = Wd^T @ hidT       (stationary = 128x128 Wd block, moving = hT [128, C])

All inputs are cast to bf16 on the host (halves weight DMA, PE runs at full
bf16 rate); accumulation is fp32 in PSUM and the output is fp32.

DMA design (what actually matters on TRN2):
- Each dma_start occupies its issuing engine ~0.6us and rings are FIFO, so
  use few, large (~1-2MB) transfers.
- The Scalar engine must stay DMA-free: its instruction stream also carries
  the sigmoids, and queued DMA triggers would block them (observed 21us
  pipeline stall).
- Two parallel rings: Sync (HWDGE) carries wg + half of wd; GpSimd (SWDGE)
  carries x, wu, the other half of wd. Within a ring, FIFO order makes the
  down-projection stream naturally after the phase-1 weights.
- Host packs weights as [128, I/128 * D] with free index i*D + d*128 + q so
  any block of i-slices is one per-partition-contiguous DMA, and each
  phase-1 step only depends on its own 1MB block.
"""

import sys

import numpy as np

try:
    import concourse  # noqa: F401
except ImportError:  # grading env fallback
    sys.path.insert(0, "/opt/trn_rl_repo")

import ml_dtypes

T, E, D, I = 2048, 8, 1024, 2048
ND = D // 128  # 8 contraction tiles for gate/up
NI = I // 128  # 16 contraction tiles for down


_PROGRAM_CACHE = {}


def _build_program(C):
    """Build + compile the per-core Bass program for token capacity C."""
    import concourse.bass as bass  # noqa: F401
    import concourse.mybir as mybir
    import concourse.tile as tile
    from concourse import bacc

    BF = mybir.dt.bfloat16
    F32 = mybir.dt.float32

    nc = bacc.Bacc(
        "TRN2",
        target_bir_lowering=False,
        debug=False,
        num_devices=E,
        enable_partition_id=False,
    )
    # xT packed: [128, ND*C], partition p / slot d*C+c  <-  x[tok c, d*128+p]
    xT_d = nc.dram_tensor("xT", [128, ND * C], BF, kind="ExternalInput").ap()
    # wg/wu/wd packed: [128, NI*D], free slot i*D + d*128 + q  <-
    #   w[d*128+p, i*128+q] for wg/wu (projection [D, I])
    #   w[i*128+p, d*128+q] for wd (projection [I, D])
    wg_d = nc.dram_tensor("wg", [128, NI * D], BF, kind="ExternalInput").ap()
    wu_d = nc.dram_tensor("wu", [128, NI * D], BF, kind="ExternalInput").ap()
    wd_d = nc.dram_tensor("wd", [128, NI * D], BF, kind="ExternalInput").ap()
    outT_d = nc.dram_tensor("outT", [D, C], F32, kind="ExternalOutput").ap()

    # PSUM bank holds 2KB/partition = 512 fp32: split the moving dim if needed.
    n_chunks = -(-C // 512)
    chunks = [(n * 512, min(512, C - n * 512)) for n in range(n_chunks)]

    with tile.TileContext(nc) as tc:
        with (
            tc.tile_pool(name="xp", bufs=1) as xp,
            tc.tile_pool(name="wp", bufs=1) as wp,
            tc.tile_pool(name="hp", bufs=1) as hp,
            tc.tile_pool(name="sp", bufs=3) as sp,
            tc.tile_pool(name="op", bufs=3) as op,
            tc.tile_pool(name="pg", bufs=3, space="PSUM") as pg,
            tc.tile_pool(name="pu", bufs=3, space="PSUM") as pu,
            tc.tile_pool(name="po", bufs=2, space="PSUM") as po,
        ):
            # One queue per engine; all three contend for the ~358GB/s HBM
            # port, so each sustains only ~110-150GB/s. Handcrafted per-queue
            # FIFO schedule: x + the i0-1 bootstrap blocks land first on the
            # two earliest-starting queues, ramped block sizes after that in
            # phase-1 consumption order, the late-phase gate/up blocks are
            # prefetched on the third (gpsimd) queue, and the down-projection
            # splits across all three queue tails so it lands before phase 2.
            xT = xp.tile([128, ND * C], BF, tag="x", name="xT")
            nc.sync.dma_start(xT[:], xT_d[:, :])

            # Global consumption-order stream of 0.5MB blocks, round-robined
            # across the three queues (prefetching out of order starves the
            # critical path — queues share HBM at packet granularity). wd
            # i-slices ride interleaved right behind the g/u pair that
            # produces the hT they'll be multiplied with.
            # 0.5MB gate/up blocks round-robined across the three queues in
            # exact phase-1 consumption order (g0 u0 g1 u1 ...): every queue
            # then delivers in need-order with growing slack, and nothing
            # non-critical competes early. The rotation puts x+g0 on sync
            # (earliest ring) and u0 first on scalar's queue. wd strictly at
            # the queue tails — it is only needed for phase 2, and any wd
            # bytes in flight early steal HBM bandwidth from the phase-1
            # stream (all queues share the ~350GB/s port).
            stream = []
            for k in range(NI // 2):
                stream.append(("g", (2 * k, 2)))
                stream.append(("u", (2 * k, 2)))
            stream += [("wd", (0, 6)), ("wd", (6, 5)), ("wd", (11, 5))]
            qs = [nc.sync, nc.scalar, nc.gpsimd]
            src = {"g": wg_d, "u": wu_d, "wd": wd_d}
            smap = {"g": [None] * NI, "u": [None] * NI, "wd": [None] * NI}
            for n, (proj, (b0, nb)) in enumerate(stream):
                t = wp.tile(
                    [128, nb * D], BF, tag=f"w{proj}{b0}", name=f"w{proj}{b0}"
                )
                # bootstrap swap: the first gate block must not queue behind
                # x on sync — it goes first on scalar's (otherwise idle)
                # queue; the first up block rides sync behind x.
                eng = qs[n % 3]
                if n == 0:
                    eng = nc.scalar
                elif n == 1:
                    eng = nc.sync
                eng.dma_start(t[:], src[proj][:, bass.ds(b0 * D, nb * D)])
                for i in range(b0, b0 + nb):
                    smap[proj][i] = (t, i - b0)

            def wslice(proj, i, d):
                t, loc = smap[proj][i]
                return t[:, bass.ds(loc * D + d * 128, 128)]

            # Phase 1: hidT[i] = silu(Wg^T x^T) * (Wu^T x^T), one 128-row
            # strip of the intermediate dim per iteration.
            hT = []
            for i in range(NI):
                h_t = hp.tile([128, C], BF, tag=f"h{i}", name=f"hT{i}")
                for c0, cn in chunks:
                    csl = bass.ds(c0, cn)
                    g_ps = pg.tile([128, cn], F32, tag="g", name="g_ps")
                    u_ps = pu.tile([128, cn], F32, tag="u", name="u_ps")
                    for d in range(ND):
                        xsl = bass.ds(d * C + c0, cn)
                        nc.tensor.matmul(
                            g_ps[:],
                            wslice("g", i, d),
                            xT[:, xsl],
                            start=(d == 0),
                            stop=(d == ND - 1),
                        )
                    for d in range(ND):
                        xsl = bass.ds(d * C + c0, cn)
                        nc.tensor.matmul(
                            u_ps[:],
                            wslice("u", i, d),
                            xT[:, xsl],
                            start=(d == 0),
                            stop=(d == ND - 1),
                        )
                    # silu(g) = g * sigmoid(g); each DVE mul reads at most
                    # one PSUM operand (DVE has a single PSUM read port).
                    s_sb = sp.tile([128, cn], F32, tag="s", name="s_sb")
                    nc.scalar.activation(
                        s_sb[:], g_ps[:], mybir.ActivationFunctionType.Sigmoid
                    )
                    gs_sb = sp.tile([128, cn], F32, tag="gs", name="gs_sb")
                    nc.vector.tensor_mul(gs_sb[:], s_sb[:], g_ps[:])
                    nc.vector.tensor_mul(h_t[:, csl], gs_sb[:], u_ps[:])
                hT.append(h_t)

            # Phase 2: outT[dstrip] = Wd^T @ hidT, accumulated over all 16
            # intermediate strips.
            for dd in range(ND):
                dsl = bass.ds(dd * 128, 128)
                for c0, cn in chunks:
                    csl = bass.ds(c0, cn)
                    o_ps = po.tile([128, cn], F32, tag="o", name="o_ps")
                    for i in range(NI):
                        nc.tensor.matmul(
                            o_ps[:],
                            wslice("wd", i, dd),
                            hT[i][:, csl],
                            start=(i == 0),
                            stop=(i == NI - 1),
                        )
                    o_sb = op.tile([128, cn], F32, tag="ob", name="o_sb")
                    nc.vector.tensor_copy(o_sb[:], o_ps[:])
                    nc.sync.dma_start(outT_d[dsl, csl], o_sb[:])

    nc.compile()
    return nc


def _get_program(C):
    if C not in _PROGRAM_CACHE:
        _PROGRAM_CACHE[C] = _build_program(C)
    return _PROGRAM_CACHE[C]


def _run(nc, in_maps, trace=False):
    from concourse.bass_utils import run_bass_kernel_spmd

    return run_bass_kernel_spmd(nc, in_maps, core_ids=list(range(E)), trace=trace)


def _pack_w(w, transpose):
    # -> [128, NI*D] bf16, free slot i*D + d*128 + q
    # transpose=True:  w is [D, I] (wg/wu), block (i,d) = w[d*128:+128, i*128:+128]
    # transpose=False: w is [I, D] (wd),   block (i,d) = w[i*128:+128, d*128:+128]
    if transpose:
        b = w.reshape(ND, 128, NI, 128).transpose(1, 2, 0, 3)  # p, i, d, q
    else:
        b = w.reshape(NI, 128, ND, 128).transpose(1, 0, 2, 3)  # p, i, d, q
    return np.ascontiguousarray(b.reshape(128, NI * D)).astype(ml_dtypes.bfloat16)


def kernel(x, expert_indices, w_gate, w_up, w_down, _trace=False, _results=None):
    x = np.asarray(x)
    idx = np.asarray(expert_indices).astype(np.int64)
    counts = np.bincount(idx, minlength=E)
    C = int(max(128, -(-counts.max() // 4) * 4))

    nc = _get_program(C)

    order = np.argsort(idx, kind="stable")
    starts = np.zeros(E + 1, dtype=np.int64)
    np.cumsum(counts, out=starts[1:])

    bf16 = ml_dtypes.bfloat16
    in_maps = []
    for e in range(E):
        toks = order[starts[e] : starts[e + 1]]
        # xT packed: [128, ND*C]; [p, d*C+c] = x[tok c, d*128+p]
        xTg = np.zeros((128, ND, C), dtype=bf16)
        xTg[:, :, : len(toks)] = (
            x[toks].astype(bf16).T.reshape(ND, 128, len(toks)).transpose(1, 0, 2)
        )
        in_maps.append(
            {
                "xT": xTg.reshape(128, ND * C),
                "wg": _pack_w(w_gate[e], True),
                "wu": _pack_w(w_up[e], True),
                "wd": _pack_w(w_down[e], False),
            }
        )

    res = _run(nc, in_maps, trace=_trace)
    if _results is not None:
        _results.append(res)

    out = np.zeros((T, D), dtype=np.float32)
    for e in range(E):
        toks = order[starts[e] : starts[e + 1]]
        outT = res.results[e]["outT"]  # [D, C] fp32
        out[toks] = outT[:, : len(toks)].T
    return out
